# revision 34
# baseline (speedup 1.0000x reference)
"""MoE (top-2 of 8 experts) Trainium2 kernel, expert-parallel across 8 NeuronCores.

Strategy (pure-GEMM device kernel, ~254us vs 548us naive / 262us prior):
  - Host: gate (fp32, exact top-2 routing), then pack the 8192 routed
    (token, expert) pairs into 8 cores x 1046 columns using a two-segment
    layout: every core runs segment A (CA=545 cols) against weight set A
    and segment B (CB=501 cols) against weight set B. Experts are assigned
    to (core, segment) slots by load rank: the 3 hottest experts take two
    A-slots each (split across a core pair), ranks 4-5 take A+B on a
    single core, the 3 coldest take two B-slots each. For the fixed-seed
    routing (max load 1090 = 2x545) this covers every pair with zero
    spill, cutting per-core columns from 1090 (max expert load, the
    single-segment SPMD floor) to 1046 — ~9us less PE time. Tokens beyond
    slot capacity (never, for this seed) fall back to an exact host FFN.
  - Device (identical SPMD program; per-core weight CONTENT differs):
    two dense GEMM phases, fp16 operands, fp32 PSUM accumulation:
      mm1: hT[f, t] = gelu(W1x.T @ xT + b1x)  (x = A or B by column)
      mm2: y[d, t]  = W2x.T @ hT
    Columns are processed in psum chunks (479, 66, 501); w1a stays fully
    SBUF-resident while w1b/w2a/w2b stream (tag-rotated tiles, prefetched
    2 slabs / 1 dtile ahead on the sync+scalar queues).
  - Startup: the critical DMA set (segment-A xT as per-dt slabs on sync,
    w1a slabs 0-3 as half-slabs on scalar, biases + slabs 4-5 on gpsimd)
    is split so range-granular deps let the first mm1 chain start
    DMA-paced at ~10us, overlapping the PE p-state ramp; a few warmup
    matmuls on zeros open the HAM activity window from ~8us. Remaining
    w1a slabs stream in-loop on scalar, four ftiles ahead.
  - mm1's 66-col A-tail chains are interleaved with the 501-col B chains
    per ftile so the ACT engine (gelu) never gates the PE. mm2 runs
    chunk-major within each dtile so each chunk's drain (DVE/ACT copy +
    out DMA) overlaps the next chunk's chain; the final dtile ends on the
    66-col chunk so only ~1us of drain is exposed after the last matmul.
  - Host: out[toks] += w * y_segment.T per (core, segment), plus the
    (combine-weight @ b2) term; this is the unshard/combine step.

Only the top-2 experts per token are ever computed (masked terms of the
reference are exactly zero), cutting FLOPs 4x vs the dense formulation.
fp8/DoubleRow was measured (e4m3 sim): rel err 5.4e-2 vs the 2e-2 gate ->
not usable. ~7.5us runtime prologue and ~4us end-barrier are fixed.
"""

import math
import sys

for _p in ("/opt/trn_rl_repo", "/root/.axon_site/_ro/trn_rl_repo"):
    if _p not in sys.path:
        sys.path.append(_p)

import numpy as np

from contextlib import ExitStack

import concourse.bass as bass
import concourse.mybir as mybir
import concourse.tile as tile
from concourse import bacc
from concourse.bass_utils import run_bass_kernel_spmd

# Problem shapes (nn_MixtureOfExperts_45243185496830)
B, S, D, E, TOPK = 2, 2048, 1024, 8, 2
DFF = 4 * D
T = B * S            # 4096 tokens
P = 128
NCORES = 8

# Two-segment column layout: CA cols of expert A + CB cols of expert B per
# core. 2*CA must cover the max expert load (1090 for the fixed seed).
CA, CB = 545, 501
CAP2 = CA + CB       # 1046 columns per core
HA = 479             # xta/xtb param split (= first psum chunk)
HB = CAP2 - HA       # 567: [0:66) = A-tail, [66:567) = B segment
NWARM = 5            # PE warmup matmuls (p-state ramp)

F32 = mybir.dt.float32
F16 = mybir.dt.float16


def build_model():
    nc = bacc.Bacc(None, target_bir_lowering=False)

    # [d_in, dt, t] in two column blocks (chunk-a cols, tail+B cols)
    xta_ext = nc.declare_dram_parameter("xta", [P, D // P, HA], F16, isOutput=False)
    xtb_ext = nc.declare_dram_parameter("xtb", [P, D // P, HB], F16, isOutput=False)
    # [ft, d_in, dt, f_in]
    w1a_ext = nc.declare_dram_parameter(
        "w1a", [DFF // P, P, D // P, P], F16, isOutput=False
    )
    w1b_ext = nc.declare_dram_parameter(
        "w1b", [DFF // P, P, D // P, P], F16, isOutput=False
    )
    b1a_ext = nc.declare_dram_parameter("b1a", [P, DFF // P], F32, isOutput=False)
    b1b_ext = nc.declare_dram_parameter("b1b", [P, DFF // P], F32, isOutput=False)
    # [dt, f_in, ft, d_in]
    w2a_ext = nc.declare_dram_parameter(
        "w2a", [D // P, P, DFF // P, P], F16, isOutput=False
    )
    w2b_ext = nc.declare_dram_parameter(
        "w2b", [D // P, P, DFF // P, P], F16, isOutput=False
    )
    out_ext = nc.declare_dram_parameter("out", [D // P, P, CAP2], F16, isOutput=True)

    with tile.TileContext(nc) as tc, ExitStack() as ctx:
        const = ctx.enter_context(tc.tile_pool(name="const", bufs=1))
        xpool = ctx.enter_context(tc.tile_pool(name="xp", bufs=1))
        hpool = ctx.enter_context(tc.tile_pool(name="hp", bufs=1))
        w1pool = ctx.enter_context(tc.tile_pool(name="w1p", bufs=1))
        w1bpool = ctx.enter_context(tc.tile_pool(name="w1bp", bufs=4))
        w2apool = ctx.enter_context(tc.tile_pool(name="w2ap", bufs=2))
        w2bpool = ctx.enter_context(tc.tile_pool(name="w2bp", bufs=2))
        ypool = ctx.enter_context(tc.tile_pool(name="yp", bufs=2))
        # psum tags are shared between mm1 and mm2 so the rotation double-
        # buffers both phases out of the same 6 banks
        ps = ctx.enter_context(tc.tile_pool(name="ps", bufs=2, space="PSUM"))

        w1a_sb = w1pool.tile([P, DFF // P, D // P, P], F16, name="w1a_sb")
        xta_sb = xpool.tile([P, D // P, HA], F16, name="xta_sb")
        xtb_sb = xpool.tile([P, D // P, HB], F16, name="xtb_sb")
        b1a_sb = const.tile([P, DFF // P], F32, name="b1a_sb")
        b1b_sb = const.tile([P, DFF // P], F32, name="b1b_sb")
        # STARTUP-CRITICAL set, ordered so the ft=0 accumulation chain can
        # begin as early as possible (DMA queues share the 16 rings, so
        # ORDER is what matters). Segment-A xT ships as 8 per-dt slabs on
        # sync in consumption order, w1a slabs 0-3 as half-slabs on scalar
        # (all shapes keep >=1KB per partition line) — range-granular deps
        # let each matmul start as soon as its own slab piece is in, so
        # the PE runs DMA-paced from ~10us. Everything else (A-tail/B
        # block, w1b, w2) is gated behind so it cannot steal HBM bandwidth
        # from the critical window.
        for dt in range(D // P):
            nc.sync.dma_start(xta_sb[:, dt], xta_ext[:, dt])
        for ft in range(4):
            nc.scalar.dma_start(w1a_sb[:, ft, :4], w1a_ext[ft, :, :4])
            nc.scalar.dma_start(w1a_sb[:, ft, 4:], w1a_ext[ft, :, 4:])
        nc.gpsimd.dma_start(b1a_sb, b1a_ext[:])
        nc.gpsimd.dma_start(b1b_sb, b1b_ext[:])
        for ft in range(4, 6):
            nc.gpsimd.dma_start(w1a_sb[:, ft], w1a_ext[ft])

        # ---- PE warmup: dummy matmuls on zeros so the HAM activity window
        # starts opening during the preamble; the real stream begins as soon
        # as the first w1a pieces + xt slabs land and is DMA-paced while the
        # p-state finishes ramping
        warm_sb = const.tile([P, 512], F16, name="warm_sb")
        nc.vector.memset(warm_sb, 0.0)
        psw = ps.tile([P, 512], F32, tag="psw", name="psw", bufs=1)
        for _ in range(NWARM):
            nc.tensor.matmul(psw[:, :], lhsT=warm_sb[:, :P], rhs=warm_sb[:, :],
                             start=True, stop=True)

        # ---- mm1 pass 1: hT[:, :HA] = gelu(W1a.T @ xtA + b1a) ----
        w1b_q = []
        w2a_pre, w2b_pre = [], []
        hT = hpool.tile([P, DFF // P, CAP2], F16, name="hT")
        for ft in range(DFF // P):
            # remaining w1a slabs stream on scalar, four ftiles ahead;
            # in-loop triggers there are naturally activation-paced
            nxt = ft + 4
            if 6 <= nxt < DFF // P:
                nc.scalar.dma_start(w1a_sb[:, nxt], w1a_ext[nxt])
            ps_ = ps.tile([P, HA], F32, tag="ps0", name="ps0")
            for dt in range(D // P):
                nc.tensor.matmul(
                    ps_[:, :],
                    lhsT=w1a_sb[:, ft, dt, :],
                    rhs=xta_sb[:, dt, :],
                    start=(dt == 0),
                    stop=(dt == D // P - 1),
                )
            nc.scalar.activation(
                out=hT[:, ft, :HA],
                in_=ps_[:, :],
                func=mybir.ActivationFunctionType.Gelu,
                bias=b1a_sb[:, ft : ft + 1],
                scale=1.0,
            )
            if ft < 4:
                # ramp fillers: keep the HAM activity window open while the
                # early ftiles run DMA-paced, so the p-state keeps climbing
                # through supply stalls; free when DMA is the constraint
                for _ in range(2):
                    nc.tensor.matmul(psw[:, :], lhsT=warm_sb[:, :P],
                                     rhs=warm_sb[:, :], start=True, stop=True)
            if ft == 28:
                # w2 prefetch: both dtile-0/1 slab pairs stream on scalar
                # (free after the in-loop w1a slabs end) well before mm2
                for k in range(2):
                    ta = w2apool.tile([P, DFF // P, P], F16, tag="w2a",
                                      name="w2a")
                    nc.scalar.dma_start(ta, w2a_ext[k])
                    w2a_pre.append(ta)
                    tb = w2bpool.tile([P, DFF // P, P], F16, tag="w2b",
                                      name="w2b")
                    nc.scalar.dma_start(tb, w2b_ext[k])
                    w2b_pre.append(tb)
            if ft == 26:
                # gate the A-tail/B column block (per-dt slabs) and the
                # first two w1b slabs on pass 1 reaching ftile 26: the
                # throwaway copy gives their DMA triggers a WAR dependency,
                # so these flow in the tail of pass 1 instead of starving
                # the critical window early on
                nc.vector.tensor_copy(
                    out=xtb_sb[:, 0, :256], in_=hT[:, 26, :256]
                )
                for dt in range(D // P):
                    nc.sync.dma_start(xtb_sb[:, dt], xtb_ext[:, dt])
                for k in range(3):
                    t = w1bpool.tile([P, D // P, P], F16, tag="w1b", name="w1b")
                    nc.sync.dma_start(t, w1b_ext[k])
                    w1b_q.append(t)

        # ---- mm1 pass 2: B segment (w1b streamed) + 66-col A-tail,
        # interleaved per ftile so ACT keeps pace with the PE ----
        for ft in range(DFF // P):
            if ft + 3 < DFF // P:
                t = w1bpool.tile([P, D // P, P], F16, tag="w1b", name="w1b")
                nc.sync.dma_start(t, w1b_ext[ft + 3])
                w1b_q.append(t)
            w1bt = w1b_q[ft]
            ps2_ = ps.tile([P, CB], F32, tag="ps2", name="ps2")
            for dt in range(D // P):
                nc.tensor.matmul(
                    ps2_[:, :],
                    lhsT=w1bt[:, dt, :],
                    rhs=xtb_sb[:, dt, CA - HA :],
                    start=(dt == 0),
                    stop=(dt == D // P - 1),
                )
            nc.scalar.activation(
                out=hT[:, ft, CA:],
                in_=ps2_[:, :],
                func=mybir.ActivationFunctionType.Gelu,
                bias=b1b_sb[:, ft : ft + 1],
                scale=1.0,
            )
            ps1_ = ps.tile([P, CA - HA], F32, tag="ps1", name="ps1")
            for dt in range(D // P):
                nc.tensor.matmul(
                    ps1_[:, :],
                    lhsT=w1a_sb[:, ft, dt, :],
                    rhs=xtb_sb[:, dt, : CA - HA],
                    start=(dt == 0),
                    stop=(dt == D // P - 1),
                )
            nc.scalar.activation(
                out=hT[:, ft, HA:CA],
                in_=ps1_[:, :],
                func=mybir.ActivationFunctionType.Gelu,
                bias=b1a_sb[:, ft : ft + 1],
                scale=1.0,
            )

        # ---- mm2: y[d_in, t] accumulated over all 32 ftiles ----
        # chunk-major within each dtile: each chunk's psum chain drains
        # (copy + out DMA) while the next chunk computes. The last dtile
        # ends on the 66-col chunk so the post-matmul drain is tiny.
        for dt in range(D // P):
            if dt < 2:
                w2at, w2bt = w2a_pre[dt], w2b_pre[dt]
            else:
                w2at = w2apool.tile([P, DFF // P, P], F16, tag="w2a", name="w2a")
                nc.scalar.dma_start(w2at, w2a_ext[dt])
                w2bt = w2bpool.tile([P, DFF // P, P], F16, tag="w2b", name="w2b")
                nc.sync.dma_start(w2bt, w2b_ext[dt])
            last = dt == D // P - 1
            y = ypool.tile([P, CAP2], F16, tag="y", name="y")
            chunks = [
                (0, HA, w2at, "ps0"),
                (HA, CA, w2at, "ps1"),
                (CA, CAP2, w2bt, "ps2"),
            ]
            if last:
                # 501 first (its ACT drain hides under the 512 chain),
                # tiny 66-col chunk last
                chunks = [chunks[2], chunks[0], chunks[1]]
            for c0, c1, w2t, tag in chunks:
                ps_ = ps.tile([P, c1 - c0], F32, tag=tag, name=tag)
                for ft in range(DFF // P):
                    nc.tensor.matmul(
                        ps_[:, :],
                        lhsT=w2t[:, ft, :],
                        rhs=hT[:, ft, c0:c1],
                        start=(ft == 0),
                        stop=(ft == DFF // P - 1),
                    )
                # drain this chunk while the next chunk's chain runs; on
                # the last dtile spread copies over DVE/ACT and DMAs over
                # two queues so the final exposure is just the 66-col piece
                if last and tag == "ps2":
                    nc.scalar.activation(
                        out=y[:, c0:c1], in_=ps_[:, :],
                        func=mybir.ActivationFunctionType.Copy,
                    )
                    nc.scalar.dma_start(out_ext[dt][:, c0:c1], y[:, c0:c1])
                else:
                    nc.vector.tensor_copy(out=y[:, c0:c1], in_=ps_[:, :])
                    nc.sync.dma_start(out_ext[dt][:, c0:c1], y[:, c0:c1])

    nc.compile()
    return nc


_NC = None

# test harness hooks: set TRACE=True before calling kernel() to capture an
# NTFF profile; the BassKernelResults lands in LAST_RESULTS.
TRACE = False
LAST_RESULTS = None


def _get_model():
    global _NC
    if _NC is None:
        _NC = build_model()
    return _NC


def _route(x2, Wg, bg):
    """Host-side gate: exact fp32 top-2 routing (matches jax.lax.top_k)."""
    logits = x2 @ Wg + bg                      # [T, E] fp32
    order = np.argsort(-logits, axis=1, kind="stable")  # top_k tie-break: first idx
    i1, i2 = order[:, 0], order[:, 1]
    l1 = logits[np.arange(T), i1]
    l2 = logits[np.arange(T), i2]
    # softmax over the two selected logits (computed in f64, cast back)
    z = np.exp(np.float64(l2) - np.float64(l1))
    w1 = (1.0 / (1.0 + z)).astype(np.float32)
    w2 = (z / (1.0 + z)).astype(np.float32)
    return i1, i2, w1, w2


def make_assignment(loads):
    """Rank-based (core, segment) slot assignment: the 3 hottest experts
    take two A-slots each (core pairs), ranks 4-5 take A+B on one core,
    the 3 coldest take two B-slots each."""
    order = np.argsort(-np.asarray(loads), kind="stable")
    slotA = [0] * NCORES
    slotB = [0] * NCORES
    pairs = ((0, 1), (2, 3), (4, 5))
    for r in range(3):
        for c in pairs[r]:
            slotA[c] = int(order[r])
    slotA[6] = slotB[6] = int(order[3])
    slotA[7] = slotB[7] = int(order[4])
    for i in range(3):
        for c in pairs[i]:
            slotB[c] = int(order[5 + i])
    return slotA, slotB


def make_in_maps(x2, W1, b1, W2, b2, Wg, bg):
    i1, i2, w1, w2 = _route(x2, Wg, bg)
    exp_toks, exp_wts = [], []
    for e in range(E):
        sel1 = i1 == e
        sel2 = i2 == e
        toks = np.nonzero(sel1 | sel2)[0]
        exp_toks.append(toks)
        exp_wts.append(
            np.where(sel1[toks], w1[toks], w2[toks]).astype(np.float32)
        )
    loads = [len(t) for t in exp_toks]
    slotA, slotB = make_assignment(loads)

    # dole each expert's tokens out to its slots in core order; leftovers
    # (never, for the fixed-seed routing) spill to the exact host FFN
    slots_of = {e: [] for e in range(E)}
    for c in range(NCORES):
        slots_of[slotA[c]].append((c, 0))
        slots_of[slotB[c]].append((c, 1))
    seg = [[None, None] for _ in range(NCORES)]
    spills = []
    for e in range(E):
        toks, wts = exp_toks[e], exp_wts[e]
        pos = 0
        for c, s in slots_of[e]:
            cap = CA if s == 0 else CB
            n = min(cap, len(toks) - pos)
            seg[c][s] = (toks[pos : pos + n], wts[pos : pos + n], e)
            pos += n
        if pos < len(toks):
            spills.append((e, toks[pos:], wts[pos:]))

    r1cache, r2cache, bcache = {}, {}, {}

    def relay1(e):
        if e not in r1cache:
            r1cache[e] = np.ascontiguousarray(
                W1[e].reshape(D // P, P, DFF // P, P)
                .transpose(2, 1, 0, 3)
                .astype(np.float16)
            )
        return r1cache[e]

    def relay2(e):
        if e not in r2cache:
            r2cache[e] = np.ascontiguousarray(
                W2[e].reshape(DFF // P, P, D // P, P)
                .transpose(2, 1, 0, 3)
                .astype(np.float16)
            )
        return r2cache[e]

    def relayb(e):
        if e not in bcache:
            bcache[e] = np.ascontiguousarray(b1[e].reshape(DFF // P, P).T)
        return bcache[e]

    in_maps, metas = [], []
    for c in range(NCORES):
        tA, wA, eA = seg[c][0]
        tB, wB, eB = seg[c][1]
        xg = np.zeros((CAP2, D), np.float16)
        xg[: len(tA)] = x2[tA]
        xg[CA : CA + len(tB)] = x2[tB]
        xt = xg.T.reshape(D // P, P, CAP2).transpose(1, 0, 2)
        m = {
            "xta": np.ascontiguousarray(xt[:, :, :HA]),
            "xtb": np.ascontiguousarray(xt[:, :, HA:]),
            "w1a": relay1(eA),
            "w1b": relay1(eB),
            "b1a": relayb(eA),
            "b1b": relayb(eB),
            "w2a": relay2(eA),
            "w2b": relay2(eB),
        }
        in_maps.append(m)
        metas.append((tA, wA, tB, wB))
    # dense combine weights for the b2 term
    wdense = np.zeros((T, E), np.float32)
    ar = np.arange(T)
    wdense[ar, i1] = w1
    wdense[ar, i2] = w2
    return in_maps, metas, spills, wdense


_erf = np.vectorize(math.erf)


def _host_ffn(x, W1e, b1e, W2e):
    """Exact-FFN fallback for tokens beyond slot capacity (normally unused)."""
    h = x.astype(np.float64) @ W1e.astype(np.float64) + b1e.astype(np.float64)
    h = 0.5 * h * (1.0 + _erf(h / np.sqrt(2.0)))
    return h @ W2e.astype(np.float64)


def kernel(x, W1, b1, W2, b2, Wg, bg):
    x = np.ascontiguousarray(np.asarray(x, dtype=np.float32))
    W1 = np.ascontiguousarray(np.asarray(W1, dtype=np.float32))
    b1 = np.ascontiguousarray(np.asarray(b1, dtype=np.float32))
    W2 = np.ascontiguousarray(np.asarray(W2, dtype=np.float32))
    b2 = np.ascontiguousarray(np.asarray(b2, dtype=np.float32))
    Wg = np.asarray(Wg, dtype=np.float32)
    bg = np.asarray(bg, dtype=np.float32)

    x2 = x.reshape(T, D)
    in_maps, metas, spills, wdense = make_in_maps(x2, W1, b1, W2, b2, Wg, bg)

    nc = _get_model()
    global LAST_RESULTS
    res = run_bass_kernel_spmd(
        nc, in_maps, core_ids=list(range(NCORES)), trace=TRACE
    )
    LAST_RESULTS = res

    out = (wdense @ b2).astype(np.float32)             # [T, D] b2 term
    for c in range(NCORES):
        tA, wA, tB, wB = metas[c]
        y = res.results[c]["out"].reshape(D, CAP2)     # [d, t] fp16
        if len(tA):
            out[tA] += wA[:, None] * y[:, : len(tA)].T.astype(np.float32)
        if len(tB):
            out[tB] += wB[:, None] * y[:, CA : CA + len(tB)].T.astype(np.float32)
    for e, toks, wts in spills:
        ys = _host_ffn(x2[toks], W1[e], b1[e], W2[e])
        out[toks] += wts[:, None] * ys.astype(np.float32)
    return out.reshape(B, S, D)


if __name__ == "__main__":
    build_model()
    print("model built ok")


# revision 35
# speedup vs baseline: 1.0082x; 1.0082x over previous
"""MoE (top-2 of 8 experts) Trainium2 kernel, expert-parallel across 8 NeuronCores.

Strategy (pure-GEMM device kernel, ~254us vs 548us naive / 262us prior):
  - Host: gate (fp32, exact top-2 routing), then pack the 8192 routed
    (token, expert) pairs into 8 cores x 1046 columns using a two-segment
    layout: every core runs segment A (CA=545 cols) against weight set A
    and segment B (CB=501 cols) against weight set B. Experts are assigned
    to (core, segment) slots by load rank: the 3 hottest experts take two
    A-slots each (split across a core pair), ranks 4-5 take A+B on a
    single core, the 3 coldest take two B-slots each. For the fixed-seed
    routing (max load 1090 = 2x545) this covers every pair with zero
    spill, cutting per-core columns from 1090 (max expert load, the
    single-segment SPMD floor) to 1046 — ~9us less PE time. Tokens beyond
    slot capacity (never, for this seed) fall back to an exact host FFN.
  - Device (identical SPMD program; per-core weight CONTENT differs):
    two dense GEMM phases, fp16 operands, fp32 PSUM accumulation:
      mm1: hT[f, t] = gelu(W1x.T @ xT + b1x)  (x = A or B by column)
      mm2: y[d, t]  = W2x.T @ hT
    Columns are processed in psum chunks (479, 66, 501); w1a stays fully
    SBUF-resident while w1b/w2a/w2b stream (tag-rotated tiles, prefetched
    2 slabs / 1 dtile ahead on the sync+scalar queues).
  - Startup: the critical DMA set (segment-A xT as per-dt slabs on sync,
    w1a slabs 0-3 as half-slabs on scalar, biases + slabs 4-5 on gpsimd)
    is split so range-granular deps let the first mm1 chain start
    DMA-paced at ~10us, overlapping the PE p-state ramp; a few warmup
    matmuls on zeros open the HAM activity window from ~8us. Remaining
    w1a slabs stream in-loop on scalar, four ftiles ahead.
  - mm1's 66-col A-tail chains are interleaved with the 501-col B chains
    per ftile so the ACT engine (gelu) never gates the PE. mm2 runs
    chunk-major within each dtile so each chunk's drain (DVE/ACT copy +
    out DMA) overlaps the next chunk's chain; the final dtile ends on the
    66-col chunk so only ~1us of drain is exposed after the last matmul.
  - Host: out[toks] += w * y_segment.T per (core, segment), plus the
    (combine-weight @ b2) term; this is the unshard/combine step.

Only the top-2 experts per token are ever computed (masked terms of the
reference are exactly zero), cutting FLOPs 4x vs the dense formulation.
fp8/DoubleRow was measured (e4m3 sim): rel err 5.4e-2 vs the 2e-2 gate ->
not usable. ~7.5us runtime prologue and ~4us end-barrier are fixed.
"""

import math
import sys

for _p in ("/opt/trn_rl_repo", "/root/.axon_site/_ro/trn_rl_repo"):
    if _p not in sys.path:
        sys.path.append(_p)

import numpy as np

from contextlib import ExitStack

import concourse.bass as bass
import concourse.mybir as mybir
import concourse.tile as tile
from concourse import bacc
from concourse.bass_utils import run_bass_kernel_spmd

# Problem shapes (nn_MixtureOfExperts_45243185496830)
B, S, D, E, TOPK = 2, 2048, 1024, 8, 2
DFF = 4 * D
T = B * S            # 4096 tokens
P = 128
NCORES = 8

# Two-segment column layout: CA cols of expert A + CB cols of expert B per
# core. 2*CA must cover the max expert load (1090 for the fixed seed).
CA, CB = 545, 501
CAP2 = CA + CB       # 1046 columns per core
HA = 479             # xta/xtb param split (= first psum chunk)
HB = CAP2 - HA       # 567: [0:66) = A-tail, [66:567) = B segment
NWARM = 5            # PE warmup matmuls (p-state ramp)

F32 = mybir.dt.float32
F16 = mybir.dt.float16


def build_model():
    nc = bacc.Bacc(None, target_bir_lowering=False)

    # [d_in, dt, t] in two column blocks (chunk-a cols, tail+B cols)
    xta_ext = nc.declare_dram_parameter("xta", [P, D // P, HA], F16, isOutput=False)
    xtb_ext = nc.declare_dram_parameter("xtb", [P, D // P, HB], F16, isOutput=False)
    # [ft, d_in, dt, f_in]
    w1a_ext = nc.declare_dram_parameter(
        "w1a", [DFF // P, P, D // P, P], F16, isOutput=False
    )
    w1b_ext = nc.declare_dram_parameter(
        "w1b", [DFF // P, P, D // P, P], F16, isOutput=False
    )
    b1a_ext = nc.declare_dram_parameter("b1a", [P, DFF // P], F32, isOutput=False)
    b1b_ext = nc.declare_dram_parameter("b1b", [P, DFF // P], F32, isOutput=False)
    # [dt, f_in, ft, d_in]
    w2a_ext = nc.declare_dram_parameter(
        "w2a", [D // P, P, DFF // P, P], F16, isOutput=False
    )
    w2b_ext = nc.declare_dram_parameter(
        "w2b", [D // P, P, DFF // P, P], F16, isOutput=False
    )
    out_ext = nc.declare_dram_parameter("out", [D // P, P, CAP2], F16, isOutput=True)

    with tile.TileContext(nc) as tc, ExitStack() as ctx:
        const = ctx.enter_context(tc.tile_pool(name="const", bufs=1))
        xpool = ctx.enter_context(tc.tile_pool(name="xp", bufs=1))
        hpool = ctx.enter_context(tc.tile_pool(name="hp", bufs=1))
        w1pool = ctx.enter_context(tc.tile_pool(name="w1p", bufs=1))
        w1bpool = ctx.enter_context(tc.tile_pool(name="w1bp", bufs=4))
        w2apool = ctx.enter_context(tc.tile_pool(name="w2ap", bufs=2))
        w2bpool = ctx.enter_context(tc.tile_pool(name="w2bp", bufs=2))
        ypool = ctx.enter_context(tc.tile_pool(name="yp", bufs=2))
        # psum tags are shared between mm1 and mm2 so the rotation double-
        # buffers both phases out of the same 6 banks
        ps = ctx.enter_context(tc.tile_pool(name="ps", bufs=2, space="PSUM"))

        w1a_sb = w1pool.tile([P, DFF // P, D // P, P], F16, name="w1a_sb")
        xta_sb = xpool.tile([P, D // P, HA], F16, name="xta_sb")
        xtb_sb = xpool.tile([P, D // P, HB], F16, name="xtb_sb")
        b1a_sb = const.tile([P, DFF // P], F32, name="b1a_sb")
        b1b_sb = const.tile([P, DFF // P], F32, name="b1b_sb")
        # STARTUP-CRITICAL set, ordered so the ft=0 accumulation chain can
        # begin as early as possible (DMA queues share the 16 rings, so
        # ORDER is what matters). Segment-A xT ships as 8 per-dt slabs on
        # sync in consumption order, w1a slabs 0-3 as half-slabs on scalar
        # (all shapes keep >=1KB per partition line) — range-granular deps
        # let each matmul start as soon as its own slab piece is in, so
        # the PE runs DMA-paced from ~10us. Everything else (A-tail/B
        # block, w1b, w2) is gated behind so it cannot steal HBM bandwidth
        # from the critical window.
        for dt in range(D // P):
            nc.sync.dma_start(xta_sb[:, dt], xta_ext[:, dt])
        for ft in range(4):
            nc.scalar.dma_start(w1a_sb[:, ft, :4], w1a_ext[ft, :, :4])
            nc.scalar.dma_start(w1a_sb[:, ft, 4:], w1a_ext[ft, :, 4:])
        nc.gpsimd.dma_start(b1a_sb, b1a_ext[:])
        nc.gpsimd.dma_start(b1b_sb, b1b_ext[:])
        for ft in range(4, 6):
            nc.gpsimd.dma_start(w1a_sb[:, ft], w1a_ext[ft])

        # ---- PE warmup: dummy matmuls on zeros so the HAM activity window
        # starts opening during the preamble; the real stream begins as soon
        # as the first w1a pieces + xt slabs land and is DMA-paced while the
        # p-state finishes ramping
        warm_sb = const.tile([P, 512], F16, name="warm_sb")
        nc.vector.memset(warm_sb, 0.0)
        psw = ps.tile([P, 512], F32, tag="psw", name="psw", bufs=1)
        for _ in range(NWARM):
            nc.tensor.matmul(psw[:, :], lhsT=warm_sb[:, :P], rhs=warm_sb[:, :],
                             start=True, stop=True)

        # ---- mm1 pass 1: hT[:, :HA] = gelu(W1a.T @ xtA + b1a) ----
        w1b_q = []
        w2a_pre, w2b_pre = [], []
        hT = hpool.tile([P, DFF // P, CAP2], F16, name="hT")
        for ft in range(DFF // P):
            # remaining w1a slabs stream on scalar, four ftiles ahead;
            # in-loop triggers there are naturally activation-paced
            nxt = ft + 4
            if 6 <= nxt < DFF // P:
                nc.scalar.dma_start(w1a_sb[:, nxt], w1a_ext[nxt])
            ps_ = ps.tile([P, HA], F32, tag="ps0", name="ps0")
            for dt in range(D // P):
                nc.tensor.matmul(
                    ps_[:, :],
                    lhsT=w1a_sb[:, ft, dt, :],
                    rhs=xta_sb[:, dt, :],
                    start=(dt == 0),
                    stop=(dt == D // P - 1),
                )
            nc.scalar.activation(
                out=hT[:, ft, :HA],
                in_=ps_[:, :],
                func=mybir.ActivationFunctionType.Gelu,
                bias=b1a_sb[:, ft : ft + 1],
                scale=1.0,
            )
            if ft == 28:
                # w2 prefetch: both dtile-0/1 slab pairs stream on scalar
                # (free after the in-loop w1a slabs end) well before mm2
                for k in range(2):
                    ta = w2apool.tile([P, DFF // P, P], F16, tag="w2a",
                                      name="w2a")
                    nc.scalar.dma_start(ta, w2a_ext[k])
                    w2a_pre.append(ta)
                    tb = w2bpool.tile([P, DFF // P, P], F16, tag="w2b",
                                      name="w2b")
                    nc.scalar.dma_start(tb, w2b_ext[k])
                    w2b_pre.append(tb)
            if ft == 26:
                # gate the A-tail/B column block (per-dt slabs) and the
                # first two w1b slabs on pass 1 reaching ftile 26: the
                # throwaway copy gives their DMA triggers a WAR dependency,
                # so these flow in the tail of pass 1 instead of starving
                # the critical window early on
                nc.vector.tensor_copy(
                    out=xtb_sb[:, 0, :256], in_=hT[:, 26, :256]
                )
                for dt in range(D // P):
                    nc.sync.dma_start(xtb_sb[:, dt], xtb_ext[:, dt])
                for k in range(3):
                    t = w1bpool.tile([P, D // P, P], F16, tag="w1b", name="w1b")
                    nc.sync.dma_start(t, w1b_ext[k])
                    w1b_q.append(t)

        # ---- mm1 pass 2: B segment (w1b streamed) + 66-col A-tail,
        # interleaved per ftile so ACT keeps pace with the PE ----
        for ft in range(DFF // P):
            if ft + 3 < DFF // P:
                t = w1bpool.tile([P, D // P, P], F16, tag="w1b", name="w1b")
                nc.sync.dma_start(t, w1b_ext[ft + 3])
                w1b_q.append(t)
            w1bt = w1b_q[ft]
            ps2_ = ps.tile([P, CB], F32, tag="ps2", name="ps2")
            for dt in range(D // P):
                nc.tensor.matmul(
                    ps2_[:, :],
                    lhsT=w1bt[:, dt, :],
                    rhs=xtb_sb[:, dt, CA - HA :],
                    start=(dt == 0),
                    stop=(dt == D // P - 1),
                )
            nc.scalar.activation(
                out=hT[:, ft, CA:],
                in_=ps2_[:, :],
                func=mybir.ActivationFunctionType.Gelu,
                bias=b1b_sb[:, ft : ft + 1],
                scale=1.0,
            )
            ps1_ = ps.tile([P, CA - HA], F32, tag="ps1", name="ps1")
            for dt in range(D // P):
                nc.tensor.matmul(
                    ps1_[:, :],
                    lhsT=w1a_sb[:, ft, dt, :],
                    rhs=xtb_sb[:, dt, : CA - HA],
                    start=(dt == 0),
                    stop=(dt == D // P - 1),
                )
            nc.scalar.activation(
                out=hT[:, ft, HA:CA],
                in_=ps1_[:, :],
                func=mybir.ActivationFunctionType.Gelu,
                bias=b1a_sb[:, ft : ft + 1],
                scale=1.0,
            )

        # ---- mm2: y[d_in, t] accumulated over all 32 ftiles ----
        # chunk-major within each dtile: each chunk's psum chain drains
        # (copy + out DMA) while the next chunk computes. The last dtile
        # ends on the 66-col chunk so the post-matmul drain is tiny.
        for dt in range(D // P):
            if dt < 2:
                w2at, w2bt = w2a_pre[dt], w2b_pre[dt]
            else:
                w2at = w2apool.tile([P, DFF // P, P], F16, tag="w2a", name="w2a")
                nc.scalar.dma_start(w2at, w2a_ext[dt])
                w2bt = w2bpool.tile([P, DFF // P, P], F16, tag="w2b", name="w2b")
                nc.sync.dma_start(w2bt, w2b_ext[dt])
            last = dt == D // P - 1
            y = ypool.tile([P, CAP2], F16, tag="y", name="y")
            chunks = [
                (0, HA, w2at, "ps0"),
                (HA, CA, w2at, "ps1"),
                (CA, CAP2, w2bt, "ps2"),
            ]
            if last:
                # 501 first (its ACT drain hides under the 512 chain),
                # tiny 66-col chunk last
                chunks = [chunks[2], chunks[0], chunks[1]]
            for c0, c1, w2t, tag in chunks:
                ps_ = ps.tile([P, c1 - c0], F32, tag=tag, name=tag)
                for ft in range(DFF // P):
                    nc.tensor.matmul(
                        ps_[:, :],
                        lhsT=w2t[:, ft, :],
                        rhs=hT[:, ft, c0:c1],
                        start=(ft == 0),
                        stop=(ft == DFF // P - 1),
                    )
                # drain this chunk while the next chunk's chain runs; on
                # the last dtile spread copies over DVE/ACT and DMAs over
                # two queues so the final exposure is just the 66-col piece
                if last and tag == "ps2":
                    nc.scalar.activation(
                        out=y[:, c0:c1], in_=ps_[:, :],
                        func=mybir.ActivationFunctionType.Copy,
                    )
                    nc.scalar.dma_start(out_ext[dt][:, c0:c1], y[:, c0:c1])
                else:
                    nc.vector.tensor_copy(out=y[:, c0:c1], in_=ps_[:, :])
                    nc.sync.dma_start(out_ext[dt][:, c0:c1], y[:, c0:c1])

    nc.compile()
    return nc


_NC = None

# test harness hooks: set TRACE=True before calling kernel() to capture an
# NTFF profile; the BassKernelResults lands in LAST_RESULTS.
TRACE = False
LAST_RESULTS = None


def _get_model():
    global _NC
    if _NC is None:
        _NC = build_model()
    return _NC


def _route(x2, Wg, bg):
    """Host-side gate: exact fp32 top-2 routing (matches jax.lax.top_k)."""
    logits = x2 @ Wg + bg                      # [T, E] fp32
    order = np.argsort(-logits, axis=1, kind="stable")  # top_k tie-break: first idx
    i1, i2 = order[:, 0], order[:, 1]
    l1 = logits[np.arange(T), i1]
    l2 = logits[np.arange(T), i2]
    # softmax over the two selected logits (computed in f64, cast back)
    z = np.exp(np.float64(l2) - np.float64(l1))
    w1 = (1.0 / (1.0 + z)).astype(np.float32)
    w2 = (z / (1.0 + z)).astype(np.float32)
    return i1, i2, w1, w2


def make_assignment(loads):
    """Rank-based (core, segment) slot assignment: the 3 hottest experts
    take two A-slots each (core pairs), ranks 4-5 take A+B on one core,
    the 3 coldest take two B-slots each."""
    order = np.argsort(-np.asarray(loads), kind="stable")
    slotA = [0] * NCORES
    slotB = [0] * NCORES
    pairs = ((0, 1), (2, 3), (4, 5))
    for r in range(3):
        for c in pairs[r]:
            slotA[c] = int(order[r])
    slotA[6] = slotB[6] = int(order[3])
    slotA[7] = slotB[7] = int(order[4])
    for i in range(3):
        for c in pairs[i]:
            slotB[c] = int(order[5 + i])
    return slotA, slotB


def make_in_maps(x2, W1, b1, W2, b2, Wg, bg):
    i1, i2, w1, w2 = _route(x2, Wg, bg)
    exp_toks, exp_wts = [], []
    for e in range(E):
        sel1 = i1 == e
        sel2 = i2 == e
        toks = np.nonzero(sel1 | sel2)[0]
        exp_toks.append(toks)
        exp_wts.append(
            np.where(sel1[toks], w1[toks], w2[toks]).astype(np.float32)
        )
    loads = [len(t) for t in exp_toks]
    slotA, slotB = make_assignment(loads)

    # dole each expert's tokens out to its slots in core order; leftovers
    # (never, for the fixed-seed routing) spill to the exact host FFN
    slots_of = {e: [] for e in range(E)}
    for c in range(NCORES):
        slots_of[slotA[c]].append((c, 0))
        slots_of[slotB[c]].append((c, 1))
    seg = [[None, None] for _ in range(NCORES)]
    spills = []
    for e in range(E):
        toks, wts = exp_toks[e], exp_wts[e]
        pos = 0
        for c, s in slots_of[e]:
            cap = CA if s == 0 else CB
            n = min(cap, len(toks) - pos)
            seg[c][s] = (toks[pos : pos + n], wts[pos : pos + n], e)
            pos += n
        if pos < len(toks):
            spills.append((e, toks[pos:], wts[pos:]))

    r1cache, r2cache, bcache = {}, {}, {}

    def relay1(e):
        if e not in r1cache:
            r1cache[e] = np.ascontiguousarray(
                W1[e].reshape(D // P, P, DFF // P, P)
                .transpose(2, 1, 0, 3)
                .astype(np.float16)
            )
        return r1cache[e]

    def relay2(e):
        if e not in r2cache:
            r2cache[e] = np.ascontiguousarray(
                W2[e].reshape(DFF // P, P, D // P, P)
                .transpose(2, 1, 0, 3)
                .astype(np.float16)
            )
        return r2cache[e]

    def relayb(e):
        if e not in bcache:
            bcache[e] = np.ascontiguousarray(b1[e].reshape(DFF // P, P).T)
        return bcache[e]

    in_maps, metas = [], []
    for c in range(NCORES):
        tA, wA, eA = seg[c][0]
        tB, wB, eB = seg[c][1]
        xg = np.zeros((CAP2, D), np.float16)
        xg[: len(tA)] = x2[tA]
        xg[CA : CA + len(tB)] = x2[tB]
        xt = xg.T.reshape(D // P, P, CAP2).transpose(1, 0, 2)
        m = {
            "xta": np.ascontiguousarray(xt[:, :, :HA]),
            "xtb": np.ascontiguousarray(xt[:, :, HA:]),
            "w1a": relay1(eA),
            "w1b": relay1(eB),
            "b1a": relayb(eA),
            "b1b": relayb(eB),
            "w2a": relay2(eA),
            "w2b": relay2(eB),
        }
        in_maps.append(m)
        metas.append((tA, wA, tB, wB))
    # dense combine weights for the b2 term
    wdense = np.zeros((T, E), np.float32)
    ar = np.arange(T)
    wdense[ar, i1] = w1
    wdense[ar, i2] = w2
    return in_maps, metas, spills, wdense


_erf = np.vectorize(math.erf)


def _host_ffn(x, W1e, b1e, W2e):
    """Exact-FFN fallback for tokens beyond slot capacity (normally unused)."""
    h = x.astype(np.float64) @ W1e.astype(np.float64) + b1e.astype(np.float64)
    h = 0.5 * h * (1.0 + _erf(h / np.sqrt(2.0)))
    return h @ W2e.astype(np.float64)


def kernel(x, W1, b1, W2, b2, Wg, bg):
    x = np.ascontiguousarray(np.asarray(x, dtype=np.float32))
    W1 = np.ascontiguousarray(np.asarray(W1, dtype=np.float32))
    b1 = np.ascontiguousarray(np.asarray(b1, dtype=np.float32))
    W2 = np.ascontiguousarray(np.asarray(W2, dtype=np.float32))
    b2 = np.ascontiguousarray(np.asarray(b2, dtype=np.float32))
    Wg = np.asarray(Wg, dtype=np.float32)
    bg = np.asarray(bg, dtype=np.float32)

    x2 = x.reshape(T, D)
    in_maps, metas, spills, wdense = make_in_maps(x2, W1, b1, W2, b2, Wg, bg)

    nc = _get_model()
    global LAST_RESULTS
    res = run_bass_kernel_spmd(
        nc, in_maps, core_ids=list(range(NCORES)), trace=TRACE
    )
    LAST_RESULTS = res

    out = (wdense @ b2).astype(np.float32)             # [T, D] b2 term
    for c in range(NCORES):
        tA, wA, tB, wB = metas[c]
        y = res.results[c]["out"].reshape(D, CAP2)     # [d, t] fp16
        if len(tA):
            out[tA] += wA[:, None] * y[:, : len(tA)].T.astype(np.float32)
        if len(tB):
            out[tB] += wB[:, None] * y[:, CA : CA + len(tB)].T.astype(np.float32)
    for e, toks, wts in spills:
        ys = _host_ffn(x2[toks], W1[e], b1[e], W2[e])
        out[toks] += wts[:, None] * ys.astype(np.float32)
    return out.reshape(B, S, D)


if __name__ == "__main__":
    build_model()
    print("model built ok")
